# revision 25
# baseline (speedup 1.0000x reference)
"""Trainium2 Bass kernel for the FilterAugment + 4-layer mirror-conv CNN.

Sharding: 8 cores = 4 samples x 2 H-halves. Odd cores work on an H-flipped
local frame so one SPMD program serves all cores (sample edge always at local
row 0); the host flips inputs/weights and un-flips the gathered output.

Per core:
  phase A: 4 separable gaussian blurs as dense banded-matrix matmuls
           (host-precomputed G matrices bake reflection padding), producing
           xpad [5, 266, 516] f16 in DRAM (channel 0 = nx copy), W-padded.
  phase B: all 4 conv layers fused in one chunk-major wavefront. Layer l
           processes rows [t*rc-2l, (t+1)*rc-2l) in chunk t; intermediates
           never touch DRAM. Each pair of output rows is one PSUM tile
           (M=128: parts 0:63 = couts of row r, 64:127 = couts of row r+1):
             x-part: 2 matmuls vs a K=125 (drow,tx,c) im2col tile whose tx
                     shifts are baked at DMA time,
             y-part (l>0): 15 [128Kx128M] pair matmuls against the previous
                     layer's slot tile.
           The PReLU epilogue (single ACT per pair) writes the PSUM pair
           straight into the next layer's SBUF slot tile: slot h part-A =
           row 2g, part-B = row 2g+1 -- exactly the +1-shifted dup layout
           the pair matmuls need, so no duplicated DMA loads exist at all.
           Between chunks only a 2-slot head copy + final f16 row stores.
All matmuls fp16 with fp32 PSUM accumulation. Output stored f16, host
converts to f32.
"""

import os
import numpy as np

B, H, W = 4, 512, 512
HALF = 256
PITCH = 516
SIGMAS = [4, 12, 48, 92]
KLENS = [s * 4 + 1 for s in SIGMAS]           # 17, 49, 193, 369
PADS = [(k - 1) // 2 for k in KLENS]          # 8, 24, 96, 184

_CACHE = {}


def _reflect(j, n=512):
    j = np.asarray(j)
    j = np.abs(j)
    return np.where(j > n - 1, 2 * (n - 1) - j, j)


def _build_g_matrices(gks, nrows):
    """Gf[i]: [512, 512] W-pass matrix; Gw_even/odd[i]: [512, regx]."""
    regx = nrows + 10          # x rows [-2, nrows+8): count nrows+10
    gfs, gws_even, gws_odd = [], [], []
    for g in gks:
        g = np.asarray(g, np.float64)
        k = len(g)
        p = (k - 1) // 2
        t = np.arange(k)
        gf = np.zeros((512, 512), np.float64)
        for wo in range(512):
            wi = _reflect(wo + t - p)
            np.add.at(gf, (wi, wo), g)
        gfs.append(gf.astype(np.float16))
        gwe = np.zeros((512, regx), np.float64)
        gwo = np.zeros((512, regx), np.float64)
        for j in range(regx):
            r = j - 2
            if r < 0:
                r = -r
            he = _reflect(r + t - p)
            np.add.at(gwe, (he, j), g)
            ho = 511 - _reflect((511 - r) + t - p)
            np.add.at(gwo, (ho, j), g)
        gws_even.append(gwe.astype(np.float16))
        gws_odd.append(gwo.astype(np.float16))
    return gfs, gws_even, gws_odd


def _pack_weights(w, flip):
    """w: [64, cin, 5, 5] f32 -> (wy [128,1920] or None, wxa [125,128],
    wxb [25,128]) f16. flip: reverse dy axis (odd cores).

    wy pair-packing: block (j, dx) at cols (j*5+dx)*128, [128K x 128M].
    K parts 0-63 = cin at slot position rr/2+j part-A (in-row r-2+2j);
    parts 64-127 = part-B (in-row r-1+2j). M cols 0-63 = cout of out row r,
    64-127 = cout of out row r+1.

    x-part: K part d*25+tx*5+c (drow=(d+4)%5) holds xpad[c, r+drow, tx+w].
    wxa: M-A tap (ty=drow, tx); M-B tap (ty=drow-1, tx) for drow>=1.
    wxb (vs the same im2col at position p1+1, parts 0:25 = drow 4 -> row
    r+5): M-B tap (ty=4, tx)."""
    w = np.asarray(w, np.float32)
    if flip:
        w = w[:, :, ::-1, :]
    cin = w.shape[1]
    if cin == 5:
        wyo = None
        wxsrc = w
    else:
        wy = np.zeros((128, 1920), np.float32)
        for j in range(3):
            for dx in range(5):
                blk = (j * 5 + dx) * 128
                t0 = w[:, 0:64, 2 * j, dx].T           # [cin, cout]
                wy[0:64, blk:blk + 64] = t0
                wy[64:128, blk + 64:blk + 128] = t0
                if 2 * j - 1 >= 0:
                    wy[0:64, blk + 64:blk + 128] = w[:, 0:64, 2 * j - 1, dx].T
                if 2 * j + 1 <= 4:
                    wy[64:128, blk:blk + 64] = w[:, 0:64, 2 * j + 1, dx].T
        wyo = wy.astype(np.float16)
        wxsrc = w[:, 64:69, :, :]
    wxa = np.zeros((125, 128), np.float32)
    wxb = np.zeros((25, 128), np.float32)
    for d in range(5):
        drow = (d + 4) % 5
        for tx in range(5):
            for c in range(5):
                p = d * 25 + tx * 5 + c
                wxa[p, 0:64] = wxsrc[:, c, drow, tx]
                if drow >= 1:
                    wxa[p, 64:128] = wxsrc[:, c, drow - 1, tx]
    for tx in range(5):
        for c in range(5):
            wxb[tx * 5 + c, 64:128] = wxsrc[:, c, 4, tx]
    return wyo, wxa.astype(np.float16), wxb.astype(np.float16)


def _blob16_layout(nrows):
    """(name -> (offset, shape)) for the packed f16 input blob."""
    regx = nrows + 10
    specs = [("nx16", (512, 512))]
    specs += [(f"gw{i}", (512, regx)) for i in range(4)]
    specs += [(f"gf{i}", (512, 512)) for i in range(4)]
    specs += [(f"wy{l}", (128, 1920)) for l in (1, 2, 3)]
    specs += [(f"wxa{l}", (125, 128)) for l in range(4)]
    specs += [(f"wxb{l}", (25, 128)) for l in range(4)]
    # ba: [128, 16] f16 carrying raw f32 bytes: cols 2l:2l+2 = b{l} (dup'd
    # to 128 partitions), cols 8+2l:10+2l = a{l}; bitcast to f32 on device.
    specs += [("ba", (128, 16))]
    layout, off = {}, 0
    for name, shp in specs:
        layout[name] = (off, shp)
        off += shp[0] * shp[1]
    return layout, off


def _build_program(nrows, rchunk):
    import concourse.tile as tile
    from concourse import bacc, mybir

    F16 = mybir.dt.float16
    F32 = mybir.dt.float32
    PRELU = mybir.ActivationFunctionType.Prelu

    rc = rchunk
    assert rc % 2 == 0 and rc >= 8
    REGX = nrows + 10                    # xpad rows: local x rows [-2, nrows+8)
    R = [nrows + 6 - 2 * l for l in range(4)]     # per-layer output rows
    # one small first chunk so the x load gating the conv pipeline is tiny
    # (~1MB vs 3MB), then uniform rc-sized chunks which keep the x-load
    # pipeline ahead of consumption
    bounds = [0, 8]
    while bounds[-1] + rc < R[0]:
        bounds.append(bounds[-1] + rc)
    bounds.append(R[0])
    if bounds[-1] - bounds[-2] < 4:
        bounds.pop(-2)
    T = len(bounds) - 1
    NSLOT = rc // 2 + 2                           # slot-tile positions
    # pass1 k-tile lists per blur (band limited)
    KT1 = [[kt for kt in range(4) if kt * 128 < (nrows + 8) + p]
           for p in PADS]

    nc = bacc.Bacc("TRN2", target_bir_lowering=False, debug=False, num_devices=8)

    lay16, tot16 = _blob16_layout(nrows)
    blob16_e = nc.dram_tensor("blob16", [tot16], F16, kind="ExternalInput")

    def v16(name):
        off, (r, c) = lay16[name]
        return blob16_e[off:off + r * c].rearrange("(r c) -> r c", c=c)

    nx_e = v16("nx16")
    gw_e = [v16(f"gw{i}") for i in range(4)]
    gf_e = [v16(f"gf{i}") for i in range(4)]
    wy_e = [None] + [v16(f"wy{l}") for l in (1, 2, 3)]
    wxa_e = [v16(f"wxa{l}") for l in range(4)]
    wxb_e = [v16(f"wxb{l}") for l in range(4)]
    out_e = nc.dram_tensor("out", [64, nrows, 512], F16, kind="ExternalOutput")

    # xpad split into row bands (separate DRAM tensors) so phase-B x loads
    # only depend on the bands they read, overlapping phase A's tail.
    BSTART = list(range(0, REGX, 128))
    BSIZE = [min(128, REGX - b) for b in BSTART]
    xpb = [nc.dram_tensor(f"xpad{b}", [5, BSIZE[b], PITCH], F16)
           for b in range(len(BSTART))]

    def band_segs(row0, n):
        """xpad rows [row0, row0+n) -> (band, local_start, length, pos_off)."""
        segs, r = [], row0
        while r < row0 + n:
            b = min(r // 128, len(BSTART) - 1)
            lr = r - BSTART[b]
            ln = min(BSTART[b] + BSIZE[b], row0 + n) - r
            segs.append((b, lr, ln, r - row0))
            r += ln
        return segs

    # padded-tile col fills: padded[0]=orig[2]->src 4, [1]=orig[1]->src 3,
    # [514]=orig[510]->src 512, [515]=orig[509]->src 511
    PADCOPY = ((0, 4), (1, 3), (514, 512), (515, 511))

    with tile.TileContext(nc) as tc:
        for _rep in range(int(os.environ.get("BK_REPEAT", "1"))):
            # single scope: phase A (blurs) + fused conv wavefront.
            # pass-2 of xpad bands 1/2 is deferred into the chunk loop so
            # only band 0's DRAM round trip sits on the critical path.
            with tc.tile_pool(name="nxp", bufs=1) as nxp, \
                 tc.tile_pool(name="gwp", bufs=8) as gwp, \
                 tc.tile_pool(name="gfp", bufs=1) as gfp, \
                 tc.tile_pool(name="otp", bufs=1) as otp, \
                 tc.tile_pool(name="xep", bufs=1) as xep, \
                 tc.tile_pool(name="xqp", bufs=4) as xqp, \
                 tc.tile_pool(name="wp", bufs=1) as wp, \
                 tc.tile_pool(name="xsp", bufs=2) as xsp, \
                 tc.tile_pool(name="yp", bufs=2) as yp, \
                 tc.tile_pool(name="rb", bufs=4) as rbp, \
                 tc.tile_pool(name="psc", bufs=8, space="PSUM") as pscp:
                nxt = []
                for kt in range(4):
                    t = nxp.tile([128, 512], F16, tag=f"nx{kt}")
                    nc.sync.dma_start(t[:], nx_e[kt * 128:(kt + 1) * 128, :])
                    nxt.append(t)
                mo_list = [(BSTART[b], BSIZE[b]) for b in range(len(BSTART))]
                # pass 1 for all blurs, then pass 2 band-major so band 0 of
                # every channel lands first (phase B chunk 0 depends on it).
                # gw (pass-1-critical) loads all go first on the sync queue;
                # the 2MB of gf (pass-2) loads go on the scalar queue so they
                # don't delay pass 1.
                gwt_all, gft_all = [], []
                for i in range(4):
                    gwt = {}
                    for kt in KT1[i]:
                        t = gwp.tile([128, REGX], F16, tag="gw")
                        nc.sync.dma_start(t[:], gw_e[i][kt * 128:(kt + 1) * 128, :])
                        gwt[kt] = t
                    gwt_all.append(gwt)
                for i in range(4):
                    gft = []
                    for kt in range(4):
                        t = gfp.tile([128, 512], F16, tag=f"gf{i}_{kt}")
                        nc.gpsimd.dma_start(t[:], gf_e[i][kt * 128:(kt + 1) * 128, :])
                        gft.append(t)
                    gft_all.append(gft)
                # channel 0 = nx copy (rows j=2..REGX-1 <- nx rows 0..REGX-3)
                # stage into padded-width tiles so every DMA is contiguous
                nxrows = REGX - 2          # = nrows + 8
                stg0 = []
                for kt in range(4):
                    lo = kt * 128
                    cnt = min(128, nxrows - lo)
                    if cnt <= 0:
                        break
                    st = xep.tile([128, PITCH], F16, tag=f"st{kt}")
                    nc.scalar.copy(st[0:cnt, 2:514], nxt[kt][0:cnt, :])
                    for d, s in PADCOPY:
                        nc.vector.tensor_copy(st[0:cnt, d:d + 1],
                                              st[0:cnt, s:s + 1])
                    for (b, lr, ln, po) in band_segs(lo + 2, cnt):
                        nc.sync.dma_start(xpb[b][0, lr:lr + ln, :],
                                          st[po:po + ln, :])
                    stg0.append(st)
                for j, srow in ((0, 2), (1, 1)):
                    nc.sync.dma_start(xpb[0][0, j, :], stg0[0][srow:srow + 1, :])
                outT_all = []
                for i in range(4):
                    gwt = gwt_all[i]
                    outT = []
                    for m in range(4):
                        ps = pscp.tile([128, 512], F32, tag="ps")
                        kts = KT1[i]
                        for j, kt in enumerate(kts):
                            nc.tensor.matmul(ps[:, 0:REGX],
                                             nxt[kt][:, m * 128:(m + 1) * 128],
                                             gwt[kt][:],
                                             start=(j == 0), stop=(j == len(kts) - 1))
                        ot = otp.tile([128, REGX], F16, tag=f"oT{i}_{m}")
                        nc.scalar.copy(ot[:], ps[:, 0:REGX])
                        outT.append(ot)
                    outT_all.append(outT)
                def emit_pass2_band(bi):
                    o0, osz = mo_list[bi]
                    for i in range(4):
                        ps = pscp.tile([128, 512], F32, tag="ps")
                        for kw in range(4):
                            nc.tensor.matmul(ps[0:osz, :],
                                             outT_all[i][kw][:, o0:o0 + osz],
                                             gft_all[i][kw][:],
                                             start=(kw == 0), stop=(kw == 3))
                        xt = xqp.tile([128, PITCH], F16, tag="xe")
                        nc.scalar.copy(xt[0:osz, 2:514], ps[0:osz, :])
                        for d, s in PADCOPY:
                            nc.vector.tensor_copy(xt[0:osz, d:d + 1],
                                                  xt[0:osz, s:s + 1])
                        nc.sync.dma_start(xpb[bi][i + 1, 0:osz, :], xt[0:osz, :])

                emit_pass2_band(0)

                # ------- fused wavefront over all 4 conv layers -------
                bat = wp.tile([128, 16], F16, tag="ba")
                nc.sync.dma_start(bat[:], v16("ba"))
                bt = [bat[:, 2 * l:2 * l + 2].bitcast(F32) for l in range(4)]
                at = [bat[:, 8 + 2 * l:10 + 2 * l].bitcast(F32) for l in range(4)]

                wxat, wxbt, wyt = [], [], [None]
                for l in range(4):
                    ta = wp.tile([125, 128], F16, tag=f"wxa{l}")
                    nc.sync.dma_start(ta[:], wxa_e[l])
                    wxat.append(ta)
                    tb = wp.tile([25, 128], F16, tag=f"wxb{l}")
                    nc.sync.dma_start(tb[:], wxb_e[l])
                    wxbt.append(tb)
                    if l > 0:
                        ty = wp.tile([128, 1920], F16, tag=f"wy{l}")
                        nc.sync.dma_start(ty[:], wy_e[l])
                        wyt.append(ty)

                cur_yt = [None, None, None]      # this chunk's slot tiles
                prev_lo = [0, 0, 0]
                for t in range(T):
                    w0 = max(0, bounds[t] - 6)
                    npos = bounds[t + 1] - w0
                    xt = xsp.tile([125, (rc + 6) * 512], F16, tag="x125")
                    xv = xt[:].rearrange("k (p w) -> k p w", w=512)
                    for d in range(5):
                        drow = (d + 4) % 5
                        for tx in range(5):
                            p0 = d * 25 + tx * 5
                            qeng = nc.gpsimd if (tx % 2 == 0) else nc.scalar
                            for (b, lr, ln, po) in band_segs(w0 + drow, npos):
                                qeng.dma_start(
                                    xv[p0:p0 + 5, po:po + ln, :],
                                    xpb[b][0:5, lr:lr + ln, tx:tx + 512])
                    for l in range(4):
                        lo = max(0, bounds[t] - 2 * l)
                        hi = min(R[l], bounds[t + 1] - 2 * l)
                        if hi <= lo:
                            continue
                        if l < 3:
                            ytn = yp.tile([128, NSLOT * PITCH], F16,
                                          tag=f"yt{l}")
                            if t > 0:
                                # head: pairs lo/2-2, lo/2-1 from prev tile
                                sh = (lo // 2 - 2) - prev_lo[l] // 2 + 2
                                nc.sync.dma_start(
                                    ytn[:, 0:2 * PITCH],
                                    cur_yt[l][:, sh * PITCH:(sh + 2) * PITCH])
                        ytin = cur_yt[l - 1] if l > 0 else None
                        lo_in = (max(0, bounds[t] - 2 * (l - 1)) // 2
                                 if l > 0 else 0)
                        for rr in range(0, hi - lo, 2):
                            r = lo + rr
                            g = r // 2
                            ps = pscp.tile([128, 512], F32, tag="ps")
                            p1 = r - w0
                            nc.tensor.matmul(
                                ps[:, :], wxat[l][:],
                                xt[:, p1 * 512:(p1 + 1) * 512],
                                start=True, stop=False,
                                skip_group_check=True)
                            nc.tensor.matmul(
                                ps[:, :], wxbt[l][:],
                                xt[0:25, (p1 + 1) * 512:(p1 + 2) * 512],
                                start=False, stop=(l == 0),
                                skip_group_check=True)
                            if l > 0:
                                for k15 in range(15):
                                    j, dx = divmod(k15, 5)
                                    h = (g - 1 + j) - lo_in + 2
                                    nc.tensor.matmul(
                                        ps[:, :],
                                        wyt[l][:, k15 * 128:(k15 + 1) * 128],
                                        ytin[:, h * PITCH + dx:
                                             h * PITCH + dx + 512],
                                        start=False, stop=(k15 == 14),
                                        skip_group_check=True)
                            if l < 3:
                                ho = rr // 2 + 2
                                nc.scalar.activation(
                                    ytn[:, ho * PITCH + 2: ho * PITCH + 514],
                                    ps[:, :], PRELU, bias=bt[l],
                                    scale=1.0, alpha=at[l])
                                for d, s in PADCOPY:
                                    nc.vector.tensor_copy(
                                        ytn[:, ho * PITCH + d:
                                            ho * PITCH + d + 1],
                                        ytn[:, ho * PITCH + s:
                                            ho * PITCH + s + 1])
                            else:
                                rb = rbp.tile([128, 512], F16, tag="rb16")
                                nc.scalar.activation(rb[:, :], ps[:, :],
                                                     PRELU, bias=bt[l],
                                                     scale=1.0, alpha=at[l])
                                nc.scalar.dma_start(out_e[:, r, :],
                                                    rb[0:64, :])
                                nc.gpsimd.dma_start(out_e[:, r + 1, :],
                                                    rb[64:128, :])
                        if l < 3:
                            if t == 0:
                                # slot 1 = pair g=-1 (rows -2,-1): part-A <-
                                # reflect row 2 (slot 3 A), part-B <- reflect
                                # row 1 (slot 2 B); full width incl pads
                                nc.vector.tensor_copy(
                                    ytn[0:64, PITCH:2 * PITCH],
                                    ytn[0:64, 3 * PITCH:4 * PITCH])
                                nc.vector.tensor_copy(
                                    ytn[64:128, PITCH:2 * PITCH],
                                    ytn[64:128, 2 * PITCH:3 * PITCH])
                            prev_lo[l] = lo
                            cur_yt[l] = ytn
                    if t + 1 < len(mo_list):
                        emit_pass2_band(t + 1)
    nc.compile()
    return nc


def _get_exec(nrows, rchunk):
    key = (nrows, rchunk)
    if key in _CACHE:
        return _CACHE[key]
    import jax
    import concourse.mybir as mybir
    from jax.sharding import Mesh, PartitionSpec
    from jax.experimental.shard_map import shard_map
    from concourse import bass2jax
    from concourse.bass2jax import _bass_exec_p, install_neuronx_cc_hook

    nc = _build_program(nrows, rchunk)
    install_neuronx_cc_hook()

    part_name = nc.partition_id_tensor.name if nc.partition_id_tensor else None
    in_names, out_names, out_avals, zero_shapes = [], [], [], []
    for alloc in nc.m.functions[0].allocations:
        if not isinstance(alloc, mybir.MemoryLocationSet):
            continue
        name = alloc.memorylocations[0].name
        if alloc.kind == "ExternalInput":
            if name != part_name:
                in_names.append(name)
        elif alloc.kind == "ExternalOutput":
            shape = tuple(alloc.tensor_shape)
            dtype = mybir.dt.np(alloc.dtype)
            out_names.append(name)
            out_avals.append(jax.core.ShapedArray(shape, dtype))
            zero_shapes.append((shape, dtype))
    n_params = len(in_names)
    n_outs = len(out_names)
    all_names = in_names + out_names
    if part_name is not None:
        all_names = all_names + [part_name]

    def _call_once(ins, out_bufs):
        operands = list(ins) + list(out_bufs)
        if part_name is not None:
            operands.append(bass2jax.partition_id_tensor())
        outs = _bass_exec_p.bind(
            *operands,
            out_avals=tuple(out_avals),
            in_names=tuple(all_names),
            out_names=tuple(out_names),
            lowering_input_output_aliases=(),
            sim_require_finite=True,
            sim_require_nnan=True,
            nc=nc,
        )
        return tuple(outs)

    def _body_iters(iters):
        def f(*args):
            ins = args[:n_params]
            bufs = list(args[n_params:n_params + n_outs])
            for _ in range(iters):
                bufs = list(_call_once(ins, bufs))
            return tuple(bufs)
        return f

    _body = _body_iters(1)

    devices = jax.devices()[:8]
    mesh = Mesh(np.asarray(devices), ("core",))
    in_specs = (PartitionSpec("core"),) * (n_params + n_outs)
    out_specs = (PartitionSpec("core"),) * n_outs
    donate = tuple(range(n_params, n_params + n_outs))
    sharded = jax.jit(
        shard_map(_body, mesh=mesh, in_specs=in_specs, out_specs=out_specs,
                  check_rep=False),
        donate_argnums=donate, keep_unused=True)

    def _concat_in(in_maps):
        return [np.concatenate([np.asarray(m[name]) for m in in_maps], axis=0)
                for name in in_names]

    def _concat_zeros():
        return [np.zeros((8 * s[0], *s[1:]), d) for s, d in zero_shapes]

    def run(in_maps):
        out_arrs = sharded(*_concat_in(in_maps), *_concat_zeros())
        return [
            {name: np.asarray(out_arrs[i]).reshape(8, *out_avals[i].shape)[c]
             for i, name in enumerate(out_names)}
            for c in range(8)
        ]

    def make_timer(in_maps, iters=1):
        """Returns sample() -> wall seconds of `iters` chained executions
        (each feeding its output buffers into the next, async dispatch,
        one final sync), device-resident I/O."""
        import time as _time
        dev_in = [jax.device_put(x) for x in _concat_in(in_maps)]
        fn = jax.jit(
            shard_map(_body, mesh=mesh, in_specs=in_specs,
                      out_specs=out_specs, check_rep=False),
            donate_argnums=tuple(range(n_params, n_params + n_outs)),
            keep_unused=True)
        zz = [jax.device_put(z) for z in _concat_zeros()]
        bufs = fn(*dev_in, *zz)          # compile + warm
        jax.block_until_ready(bufs)

        state = {"bufs": bufs}

        def sample():
            bufs = state["bufs"]
            t0 = _time.time()
            for _ in range(iters):
                bufs = fn(*dev_in, *bufs)
            jax.block_until_ready(bufs)
            dt = _time.time() - t0
            state["bufs"] = bufs
            return dt
        return sample

    def time_exec(in_maps, repeats=10, iters=1):
        s = make_timer(in_maps, iters=iters)
        return min(s() for _ in range(repeats)) / iters

    run.time_exec = time_exec
    run.make_timer = make_timer
    _CACHE[key] = run
    return run


def baseline_time(repeats=10, iters=1):
    """Time an (almost) empty program with the same input/output signature
    as the kernel program, to subtract dispatch/RPC/input-registration
    overhead from time_exec (the axon execute path has a per-input-byte
    cost that is launch overhead, not kernel work)."""
    if ("baseline", iters) in _CACHE:
        return _CACHE[("baseline", iters)](repeats)
    import jax
    import concourse.tile as tile
    from concourse import bacc, mybir

    F16 = mybir.dt.float16
    nrows = int(os.environ.get("BK_NROWS", HALF))
    _, tot16 = _blob16_layout(nrows)
    nc = bacc.Bacc("TRN2", target_bir_lowering=False, debug=False, num_devices=8)
    x_e = nc.dram_tensor("blob16", [tot16], F16, kind="ExternalInput")
    out_e = nc.dram_tensor("out", [64, nrows, 512], F16, kind="ExternalOutput")
    with tile.TileContext(nc) as tc:
        with tc.tile_pool(name="sb", bufs=1) as sb:
            t = sb.tile([64, 512], F16)
            nc.sync.dma_start(t[:], x_e[0:64 * 512].rearrange("(r c) -> r c",
                                                              c=512))
            t2 = sb.tile([64, 512], F16)
            nc.scalar.copy(t2[:], t[:])
            nc.sync.dma_start(out_e[:, 0, :], t2[:])
    nc.compile()
    runner = _wrap_exec(nc, iters=iters)

    import numpy as _np
    in_maps = [{"blob16": _np.zeros(tot16, _np.float16)} for _ in range(8)]
    sampler = runner(in_maps)

    def bt(reps):
        return min(sampler() for _ in range(reps)) / iters

    bt.sample = sampler
    _CACHE[("baseline", iters)] = bt
    return bt(repeats)


def _wrap_exec(nc, iters=1):
    """Minimal timed executor for an arbitrary compiled nc (used by baseline)."""
    import jax
    import concourse.mybir as mybir
    from jax.sharding import Mesh, PartitionSpec
    from jax.experimental.shard_map import shard_map
    from concourse import bass2jax
    from concourse.bass2jax import _bass_exec_p, install_neuronx_cc_hook
    install_neuronx_cc_hook()

    part_name = nc.partition_id_tensor.name if nc.partition_id_tensor else None
    in_names, out_names, out_avals, zero_shapes = [], [], [], []
    for alloc in nc.m.functions[0].allocations:
        if not isinstance(alloc, mybir.MemoryLocationSet):
            continue
        name = alloc.memorylocations[0].name
        if alloc.kind == "ExternalInput":
            if name != part_name:
                in_names.append(name)
        elif alloc.kind == "ExternalOutput":
            shape = tuple(alloc.tensor_shape)
            dtype = mybir.dt.np(alloc.dtype)
            out_names.append(name)
            out_avals.append(jax.core.ShapedArray(shape, dtype))
            zero_shapes.append((shape, dtype))
    n_params, n_outs = len(in_names), len(out_names)
    all_names = in_names + out_names + ([part_name] if part_name else [])

    def _body(*args):
        operands = list(args)
        if part_name is not None:
            operands.append(bass2jax.partition_id_tensor())
        return tuple(_bass_exec_p.bind(
            *operands, out_avals=tuple(out_avals), in_names=tuple(all_names),
            out_names=tuple(out_names), lowering_input_output_aliases=(),
            sim_require_finite=True, sim_require_nnan=True, nc=nc))

    devices = jax.devices()[:8]
    mesh = Mesh(np.asarray(devices), ("core",))
    fn = jax.jit(
        shard_map(_body, mesh=mesh,
                  in_specs=(PartitionSpec("core"),) * (n_params + n_outs),
                  out_specs=(PartitionSpec("core"),) * n_outs,
                  check_rep=False),
        donate_argnums=tuple(range(n_params, n_params + n_outs)),
        keep_unused=True)

    def timed(in_maps):
        import time as _time
        dev_in = [jax.device_put(
            np.concatenate([np.asarray(m[nm]) for m in in_maps], axis=0))
            for nm in in_names]
        zz = [jax.device_put(np.zeros((8 * s[0], *s[1:]), d))
              for s, d in zero_shapes]
        bufs = fn(*dev_in, *zz)
        jax.block_until_ready(bufs)
        state = {"bufs": bufs}

        def sample():
            bufs = state["bufs"]
            t0 = _time.time()
            for _ in range(iters):
                bufs = fn(*dev_in, *bufs)
            jax.block_until_ready(bufs)
            dt = _time.time() - t0
            state["bufs"] = bufs
            return dt
        return sample

    return timed


def _make_in_maps(inputs, nrows):
    nx = np.asarray(inputs["nx"], np.float32)        # [4, 1, 512, 512]
    gks = [np.asarray(inputs[f"gk{i}"], np.float32) for i in range(4)]
    gfs, gwe, gwo = _build_g_matrices(gks, nrows)
    packs_even = [_pack_weights(inputs[f"w{l}"], False) for l in range(4)]
    packs_odd = [_pack_weights(inputs[f"w{l}"], True) for l in range(4)]
    lay16, tot16 = _blob16_layout(nrows)

    ba = np.zeros((128, 16), np.float16)
    for l in range(4):
        b2 = np.concatenate([np.asarray(inputs[f"b{l}"], np.float32)] * 2)
        ba[:, 2 * l:2 * l + 2] = b2.view(np.float16).reshape(128, 2)
        av = np.full(128, np.asarray(inputs[f"a{l}"], np.float32).reshape(1)[0],
                     np.float32)
        ba[:, 8 + 2 * l:10 + 2 * l] = av.view(np.float16).reshape(128, 2)

    in_maps = []
    for c in range(8):
        s, half = c >> 1, c & 1
        img = nx[s, 0]
        if half:
            img = img[::-1, :]
        vals = {"nx16": np.ascontiguousarray(img).astype(np.float16)}
        gw = gwo if half else gwe
        for i in range(4):
            vals[f"gw{i}"] = gw[i]
            vals[f"gf{i}"] = gfs[i]
        packs = packs_odd if half else packs_even
        for l in (1, 2, 3):
            vals[f"wy{l}"] = packs[l][0]
        for l in range(4):
            vals[f"wxa{l}"] = packs[l][1]
            vals[f"wxb{l}"] = packs[l][2]
        vals["ba"] = ba
        blob16 = np.zeros(tot16, np.float16)
        for name, (off, shp) in lay16.items():
            v = vals[name]
            assert v.shape == shp, (name, v.shape, shp)
            blob16[off:off + shp[0] * shp[1]] = v.ravel()
        in_maps.append({"blob16": blob16})
    return in_maps


def kernel(**inputs) -> np.ndarray:
    nrows = int(os.environ.get("BK_NROWS", HALF))
    rchunk = int(os.environ.get("BK_RCHUNK", 24))
    run = _get_exec(nrows, rchunk)
    in_maps = _make_in_maps(inputs, nrows)
    results = run(in_maps)
    out = np.zeros((B, 64, H, W), np.float32)
    for c in range(8):
        s, half = c >> 1, c & 1
        o = results[c]["out"].astype(np.float32)   # [64, nrows, 512] f16
        if half:
            out[s, :, H - nrows:H, :] = o[:, ::-1, :]
        else:
            out[s, :, 0:nrows, :] = o
    return out


# revision 26
# speedup vs baseline: 1.1053x; 1.1053x over previous
"""Trainium2 Bass kernel for the FilterAugment + 4-layer mirror-conv CNN.

Sharding: 8 cores = 4 samples x 2 H-halves. Odd cores work on an H-flipped
local frame so one SPMD program serves all cores (sample edge always at local
row 0); the host flips inputs/weights and un-flips the gathered output.

Per core:
  phase A: 4 separable gaussian blurs as dense banded-matrix matmuls
           (host-precomputed G matrices bake reflection padding), producing
           xpad [5, 266, 516] f16 in DRAM (channel 0 = nx copy), W-padded.
  phase B: all 4 conv layers fused in one chunk-major wavefront. Layer l
           processes rows [t*rc-2l, (t+1)*rc-2l) in chunk t; intermediates
           never touch DRAM. Each pair of output rows is one PSUM tile
           (M=128: parts 0:63 = couts of row r, 64:127 = couts of row r+1):
             x-part: 2 matmuls vs a K=125 (drow,tx,c) im2col tile whose tx
                     shifts are baked at DMA time,
             y-part (l>0): 15 [128Kx128M] pair matmuls against the previous
                     layer's slot tile.
           The PReLU epilogue (single ACT per pair) writes the PSUM pair
           straight into the next layer's SBUF slot tile: slot h part-A =
           row 2g, part-B = row 2g+1 -- exactly the +1-shifted dup layout
           the pair matmuls need, so no duplicated DMA loads exist at all.
           Between chunks only a 2-slot head copy + final f16 row stores.
All matmuls fp16 with fp32 PSUM accumulation. Output stored f16, host
converts to f32.
"""

import os
import numpy as np

B, H, W = 4, 512, 512
HALF = 256
PITCH = 516
SIGMAS = [4, 12, 48, 92]
KLENS = [s * 4 + 1 for s in SIGMAS]           # 17, 49, 193, 369
PADS = [(k - 1) // 2 for k in KLENS]          # 8, 24, 96, 184

_CACHE = {}


def _reflect(j, n=512):
    j = np.asarray(j)
    j = np.abs(j)
    return np.where(j > n - 1, 2 * (n - 1) - j, j)


def _build_g_matrices(gks, nrows):
    """Gf[i]: [512, 512] W-pass matrix; Gw_even/odd[i]: [512, regx]."""
    regx = nrows + 10          # x rows [-2, nrows+8): count nrows+10
    gfs, gws_even, gws_odd = [], [], []
    for g in gks:
        g = np.asarray(g, np.float64)
        k = len(g)
        p = (k - 1) // 2
        t = np.arange(k)
        gf = np.zeros((512, 512), np.float64)
        for wo in range(512):
            wi = _reflect(wo + t - p)
            np.add.at(gf, (wi, wo), g)
        gfs.append(gf.astype(np.float16))
        gwe = np.zeros((512, regx), np.float64)
        gwo = np.zeros((512, regx), np.float64)
        for j in range(regx):
            r = j - 2
            if r < 0:
                r = -r
            he = _reflect(r + t - p)
            np.add.at(gwe, (he, j), g)
            ho = 511 - _reflect((511 - r) + t - p)
            np.add.at(gwo, (ho, j), g)
        gws_even.append(gwe.astype(np.float16))
        gws_odd.append(gwo.astype(np.float16))
    return gfs, gws_even, gws_odd


def _pack_weights(w, flip):
    """w: [64, cin, 5, 5] f32 -> (wy [128,1920] or None, wxa [125,128],
    wxb [25,128]) f16. flip: reverse dy axis (odd cores).

    wy pair-packing: block (j, dx) at cols (j*5+dx)*128, [128K x 128M].
    K parts 0-63 = cin at slot position rr/2+j part-A (in-row r-2+2j);
    parts 64-127 = part-B (in-row r-1+2j). M cols 0-63 = cout of out row r,
    64-127 = cout of out row r+1.

    x-part: K part d*25+tx*5+c (drow=(d+4)%5) holds xpad[c, r+drow, tx+w].
    wxa: M-A tap (ty=drow, tx); M-B tap (ty=drow-1, tx) for drow>=1.
    wxb (vs the same im2col at position p1+1, parts 0:25 = drow 4 -> row
    r+5): M-B tap (ty=4, tx)."""
    w = np.asarray(w, np.float32)
    if flip:
        w = w[:, :, ::-1, :]
    cin = w.shape[1]
    if cin == 5:
        wyo = None
        wxsrc = w
    else:
        wy = np.zeros((128, 1920), np.float32)
        for j in range(3):
            for dx in range(5):
                blk = (j * 5 + dx) * 128
                t0 = w[:, 0:64, 2 * j, dx].T           # [cin, cout]
                wy[0:64, blk:blk + 64] = t0
                wy[64:128, blk + 64:blk + 128] = t0
                if 2 * j - 1 >= 0:
                    wy[0:64, blk + 64:blk + 128] = w[:, 0:64, 2 * j - 1, dx].T
                if 2 * j + 1 <= 4:
                    wy[64:128, blk:blk + 64] = w[:, 0:64, 2 * j + 1, dx].T
        wyo = wy.astype(np.float16)
        wxsrc = w[:, 64:69, :, :]
    wxa = np.zeros((125, 128), np.float32)
    wxb = np.zeros((25, 128), np.float32)
    for d in range(5):
        drow = (d + 4) % 5
        for tx in range(5):
            for c in range(5):
                p = d * 25 + tx * 5 + c
                wxa[p, 0:64] = wxsrc[:, c, drow, tx]
                if drow >= 1:
                    wxa[p, 64:128] = wxsrc[:, c, drow - 1, tx]
    for tx in range(5):
        for c in range(5):
            wxb[tx * 5 + c, 64:128] = wxsrc[:, c, 4, tx]
    return wyo, wxa.astype(np.float16), wxb.astype(np.float16)


def _blob16_layout(nrows):
    """(name -> (offset, shape)) for the packed f16 input blob."""
    regx = nrows + 10
    specs = [("nx16", (512, 512))]
    specs += [(f"gw{i}", (512, regx)) for i in range(4)]
    specs += [(f"gf{i}", (512, 512)) for i in range(4)]
    specs += [(f"wy{l}", (128, 1920)) for l in (1, 2, 3)]
    specs += [(f"wxa{l}", (125, 128)) for l in range(4)]
    specs += [(f"wxb{l}", (25, 128)) for l in range(4)]
    # ba: [128, 16] f16 carrying raw f32 bytes: cols 2l:2l+2 = b{l} (dup'd
    # to 128 partitions), cols 8+2l:10+2l = a{l}; bitcast to f32 on device.
    specs += [("ba", (128, 16))]
    layout, off = {}, 0
    for name, shp in specs:
        layout[name] = (off, shp)
        off += shp[0] * shp[1]
    return layout, off


def _build_program(nrows, rchunk):
    import concourse.tile as tile
    from concourse import bacc, mybir

    F16 = mybir.dt.float16
    F32 = mybir.dt.float32
    PRELU = mybir.ActivationFunctionType.Prelu

    rc = rchunk
    assert rc % 2 == 0 and rc >= 8
    REGX = nrows + 10                    # xpad rows: local x rows [-2, nrows+8)
    R = [nrows + 6 - 2 * l for l in range(4)]     # per-layer output rows
    # one small first chunk so the x load gating the conv pipeline is tiny
    # (~1MB vs 3MB), then uniform rc-sized chunks which keep the x-load
    # pipeline ahead of consumption
    bounds = [0, 8]
    while bounds[-1] + rc < R[0]:
        bounds.append(bounds[-1] + rc)
    bounds.append(R[0])
    if bounds[-1] - bounds[-2] < 4:
        bounds.pop(-2)
    T = len(bounds) - 1
    NSLOT = rc // 2 + 2                           # slot-tile positions
    # pass1 k-tile lists per blur (band limited)
    KT1 = [[kt for kt in range(4) if kt * 128 < (nrows + 8) + p]
           for p in PADS]

    nc = bacc.Bacc("TRN2", target_bir_lowering=False, debug=False, num_devices=8)

    lay16, tot16 = _blob16_layout(nrows)
    blob16_e = nc.dram_tensor("blob16", [tot16], F16, kind="ExternalInput")

    def v16(name):
        off, (r, c) = lay16[name]
        return blob16_e[off:off + r * c].rearrange("(r c) -> r c", c=c)

    nx_e = v16("nx16")
    gw_e = [v16(f"gw{i}") for i in range(4)]
    gf_e = [v16(f"gf{i}") for i in range(4)]
    wy_e = [None] + [v16(f"wy{l}") for l in (1, 2, 3)]
    wxa_e = [v16(f"wxa{l}") for l in range(4)]
    wxb_e = [v16(f"wxb{l}") for l in range(4)]
    out_e = nc.dram_tensor("out", [64, nrows, 512], F16, kind="ExternalOutput")

    # xpad split into row bands (separate DRAM tensors) so phase-B x loads
    # only depend on the bands they read, overlapping phase A's tail.
    BSTART = list(range(0, REGX, 128))
    BSIZE = [min(128, REGX - b) for b in BSTART]
    xpb = [nc.dram_tensor(f"xpad{b}", [5, BSIZE[b], PITCH], F16)
           for b in range(len(BSTART))]

    def band_segs(row0, n):
        """xpad rows [row0, row0+n) -> (band, local_start, length, pos_off)."""
        segs, r = [], row0
        while r < row0 + n:
            b = min(r // 128, len(BSTART) - 1)
            lr = r - BSTART[b]
            ln = min(BSTART[b] + BSIZE[b], row0 + n) - r
            segs.append((b, lr, ln, r - row0))
            r += ln
        return segs

    # padded-tile col fills: padded[0]=orig[2]->src 4, [1]=orig[1]->src 3,
    # [514]=orig[510]->src 512, [515]=orig[509]->src 511
    PADCOPY = ((0, 4), (1, 3), (514, 512), (515, 511))

    with tile.TileContext(nc) as tc:
        for _rep in range(int(os.environ.get("BK_REPEAT", "1"))):
            # single scope: phase A (blurs) + fused conv wavefront.
            # pass-2 of xpad bands 1/2 is deferred into the chunk loop so
            # only band 0's DRAM round trip sits on the critical path.
            with tc.tile_pool(name="nxp", bufs=1) as nxp, \
                 tc.tile_pool(name="gwp", bufs=8) as gwp, \
                 tc.tile_pool(name="gfp", bufs=1) as gfp, \
                 tc.tile_pool(name="otp", bufs=1) as otp, \
                 tc.tile_pool(name="xep", bufs=1) as xep, \
                 tc.tile_pool(name="xqp", bufs=4) as xqp, \
                 tc.tile_pool(name="wp", bufs=1) as wp, \
                 tc.tile_pool(name="xsp", bufs=2) as xsp, \
                 tc.tile_pool(name="yp", bufs=2) as yp, \
                 tc.tile_pool(name="rb", bufs=4) as rbp, \
                 tc.tile_pool(name="psc", bufs=8, space="PSUM") as pscp:
                nxt = []
                for kt in range(4):
                    t = nxp.tile([128, 512], F16, tag=f"nx{kt}")
                    nc.sync.dma_start(t[:], nx_e[kt * 128:(kt + 1) * 128, :])
                    nxt.append(t)
                mo_list = [(BSTART[b], BSIZE[b]) for b in range(len(BSTART))]
                # pass 1 for all blurs, then pass 2 band-major so band 0 of
                # every channel lands first (phase B chunk 0 depends on it).
                # gw (pass-1-critical) loads all go first on the sync queue;
                # the 2MB of gf (pass-2) loads go on the scalar queue so they
                # don't delay pass 1.
                gwt_all, gft_all = [], []
                for i in range(4):
                    gwt = {}
                    for kt in KT1[i]:
                        t = gwp.tile([128, REGX], F16, tag="gw")
                        nc.sync.dma_start(t[:], gw_e[i][kt * 128:(kt + 1) * 128, :])
                        gwt[kt] = t
                    gwt_all.append(gwt)
                for i in range(4):
                    gft = []
                    for kt in range(4):
                        t = gfp.tile([128, 512], F16, tag=f"gf{i}_{kt}")
                        nc.gpsimd.dma_start(t[:], gf_e[i][kt * 128:(kt + 1) * 128, :])
                        gft.append(t)
                    gft_all.append(gft)
                # channel 0 = nx copy (rows j=2..REGX-1 <- nx rows 0..REGX-3)
                # stage into padded-width tiles so every DMA is contiguous
                nxrows = REGX - 2          # = nrows + 8
                stg0 = []
                for kt in range(4):
                    lo = kt * 128
                    cnt = min(128, nxrows - lo)
                    if cnt <= 0:
                        break
                    st = xep.tile([128, PITCH], F16, tag=f"st{kt}")
                    nc.scalar.copy(st[0:cnt, 2:514], nxt[kt][0:cnt, :])
                    for d, s in PADCOPY:
                        nc.vector.tensor_copy(st[0:cnt, d:d + 1],
                                              st[0:cnt, s:s + 1])
                    for (b, lr, ln, po) in band_segs(lo + 2, cnt):
                        nc.sync.dma_start(xpb[b][0, lr:lr + ln, :],
                                          st[po:po + ln, :])
                    stg0.append(st)
                for j, srow in ((0, 2), (1, 1)):
                    nc.sync.dma_start(xpb[0][0, j, :], stg0[0][srow:srow + 1, :])
                outT_all = []
                for i in range(4):
                    gwt = gwt_all[i]
                    outT = []
                    for m in range(4):
                        ps = pscp.tile([128, 512], F32, tag="ps")
                        kts = KT1[i]
                        for j, kt in enumerate(kts):
                            nc.tensor.matmul(ps[:, 0:REGX],
                                             nxt[kt][:, m * 128:(m + 1) * 128],
                                             gwt[kt][:],
                                             start=(j == 0), stop=(j == len(kts) - 1))
                        ot = otp.tile([128, REGX], F16, tag=f"oT{i}_{m}")
                        nc.scalar.copy(ot[:], ps[:, 0:REGX])
                        outT.append(ot)
                    outT_all.append(outT)
                def emit_pass2_band(bi):
                    o0, osz = mo_list[bi]
                    for i in range(4):
                        ps = pscp.tile([128, 512], F32, tag="ps")
                        for kw in range(4):
                            nc.tensor.matmul(ps[0:osz, :],
                                             outT_all[i][kw][:, o0:o0 + osz],
                                             gft_all[i][kw][:],
                                             start=(kw == 0), stop=(kw == 3))
                        xt = xqp.tile([128, PITCH], F16, tag="xe")
                        nc.scalar.copy(xt[0:osz, 2:514], ps[0:osz, :])
                        for d, s in PADCOPY:
                            nc.vector.tensor_copy(xt[0:osz, d:d + 1],
                                                  xt[0:osz, s:s + 1])
                        nc.sync.dma_start(xpb[bi][i + 1, 0:osz, :], xt[0:osz, :])

                emit_pass2_band(0)

                # ------- fused wavefront over all 4 conv layers -------
                bat = wp.tile([128, 16], F16, tag="ba")
                nc.sync.dma_start(bat[:], v16("ba"))
                bt = [bat[:, 2 * l:2 * l + 2].bitcast(F32) for l in range(4)]
                at = [bat[:, 8 + 2 * l:10 + 2 * l].bitcast(F32) for l in range(4)]

                wxat, wxbt, wyt = [], [], [None]
                for l in range(4):
                    ta = wp.tile([125, 128], F16, tag=f"wxa{l}")
                    nc.sync.dma_start(ta[:], wxa_e[l])
                    wxat.append(ta)
                    tb = wp.tile([25, 128], F16, tag=f"wxb{l}")
                    nc.sync.dma_start(tb[:], wxb_e[l])
                    wxbt.append(tb)
                    if l > 0:
                        ty = wp.tile([128, 1920], F16, tag=f"wy{l}")
                        nc.sync.dma_start(ty[:], wy_e[l])
                        wyt.append(ty)

                cur_yt = [None, None, None]      # this chunk's slot tiles
                prev_lo = [0, 0, 0]
                for t in range(T):
                    w0 = max(0, bounds[t] - 6)
                    npos = bounds[t + 1] - w0
                    xt = xsp.tile([125, (rc + 6) * 512], F16, tag="x125")
                    xv = xt[:].rearrange("k (p w) -> k p w", w=512)
                    for d in range(5):
                        drow = (d + 4) % 5
                        for tx in range(5):
                            p0 = d * 25 + tx * 5
                            qeng = nc.gpsimd if (tx % 2 == 0) else nc.scalar
                            for (b, lr, ln, po) in band_segs(w0 + drow, npos):
                                qeng.dma_start(
                                    xv[p0:p0 + 5, po:po + ln, :],
                                    xpb[b][0:5, lr:lr + ln, tx:tx + 512])
                    for l in range(4):
                        lo = max(0, bounds[t] - 2 * l)
                        hi = min(R[l], bounds[t + 1] - 2 * l)
                        if hi <= lo:
                            continue
                        if l < 3:
                            ytn = yp.tile([128, NSLOT * PITCH], F16,
                                          tag=f"yt{l}")
                            if t > 0:
                                # head: pairs lo/2-2, lo/2-1 from prev tile
                                sh = (lo // 2 - 2) - prev_lo[l] // 2 + 2
                                nc.sync.dma_start(
                                    ytn[:, 0:2 * PITCH],
                                    cur_yt[l][:, sh * PITCH:(sh + 2) * PITCH])
                        ytin = cur_yt[l - 1] if l > 0 else None
                        lo_in = (max(0, bounds[t] - 2 * (l - 1)) // 2
                                 if l > 0 else 0)
                        for rr in range(0, hi - lo, 2):
                            r = lo + rr
                            g = r // 2
                            ps = pscp.tile([128, 512], F32, tag="ps")
                            p1 = r - w0
                            nc.tensor.matmul(
                                ps[:, :], wxat[l][:],
                                xt[:, p1 * 512:(p1 + 1) * 512],
                                start=True, stop=False,
                                skip_group_check=True)
                            nc.tensor.matmul(
                                ps[:, :], wxbt[l][:],
                                xt[0:25, (p1 + 1) * 512:(p1 + 2) * 512],
                                start=False, stop=(l == 0),
                                skip_group_check=True)
                            if l > 0:
                                for k15 in range(15):
                                    j, dx = divmod(k15, 5)
                                    h = (g - 1 + j) - lo_in + 2
                                    nc.tensor.matmul(
                                        ps[:, :],
                                        wyt[l][:, k15 * 128:(k15 + 1) * 128],
                                        ytin[:, h * PITCH + dx:
                                             h * PITCH + dx + 512],
                                        start=False, stop=(k15 == 14),
                                        skip_group_check=True)
                            if l < 3:
                                ho = rr // 2 + 2
                                nc.scalar.activation(
                                    ytn[:, ho * PITCH + 2: ho * PITCH + 514],
                                    ps[:, :], PRELU, bias=bt[l],
                                    scale=1.0, alpha=at[l])
                                for d, s in PADCOPY:
                                    nc.vector.tensor_copy(
                                        ytn[:, ho * PITCH + d:
                                            ho * PITCH + d + 1],
                                        ytn[:, ho * PITCH + s:
                                            ho * PITCH + s + 1])
                            else:
                                rb = rbp.tile([128, 512], F16, tag="rb16")
                                nc.scalar.activation(rb[:, :], ps[:, :],
                                                     PRELU, bias=bt[l],
                                                     scale=1.0, alpha=at[l])
                                nc.scalar.dma_start(out_e[:, r, :],
                                                    rb[0:64, :])
                                nc.scalar.dma_start(out_e[:, r + 1, :],
                                                    rb[64:128, :])
                        if l < 3:
                            if t == 0:
                                # slot 1 = pair g=-1 (rows -2,-1): part-A <-
                                # reflect row 2 (slot 3 A), part-B <- reflect
                                # row 1 (slot 2 B); full width incl pads
                                nc.vector.tensor_copy(
                                    ytn[0:64, PITCH:2 * PITCH],
                                    ytn[0:64, 3 * PITCH:4 * PITCH])
                                nc.vector.tensor_copy(
                                    ytn[64:128, PITCH:2 * PITCH],
                                    ytn[64:128, 2 * PITCH:3 * PITCH])
                            prev_lo[l] = lo
                            cur_yt[l] = ytn
                    if t + 1 < len(mo_list):
                        emit_pass2_band(t + 1)
    nc.compile()
    return nc


def _get_exec(nrows, rchunk):
    key = (nrows, rchunk)
    if key in _CACHE:
        return _CACHE[key]
    import jax
    import concourse.mybir as mybir
    from jax.sharding import Mesh, PartitionSpec
    from jax.experimental.shard_map import shard_map
    from concourse import bass2jax
    from concourse.bass2jax import _bass_exec_p, install_neuronx_cc_hook

    nc = _build_program(nrows, rchunk)
    install_neuronx_cc_hook()

    part_name = nc.partition_id_tensor.name if nc.partition_id_tensor else None
    in_names, out_names, out_avals, zero_shapes = [], [], [], []
    for alloc in nc.m.functions[0].allocations:
        if not isinstance(alloc, mybir.MemoryLocationSet):
            continue
        name = alloc.memorylocations[0].name
        if alloc.kind == "ExternalInput":
            if name != part_name:
                in_names.append(name)
        elif alloc.kind == "ExternalOutput":
            shape = tuple(alloc.tensor_shape)
            dtype = mybir.dt.np(alloc.dtype)
            out_names.append(name)
            out_avals.append(jax.core.ShapedArray(shape, dtype))
            zero_shapes.append((shape, dtype))
    n_params = len(in_names)
    n_outs = len(out_names)
    all_names = in_names + out_names
    if part_name is not None:
        all_names = all_names + [part_name]

    def _call_once(ins, out_bufs):
        operands = list(ins) + list(out_bufs)
        if part_name is not None:
            operands.append(bass2jax.partition_id_tensor())
        outs = _bass_exec_p.bind(
            *operands,
            out_avals=tuple(out_avals),
            in_names=tuple(all_names),
            out_names=tuple(out_names),
            lowering_input_output_aliases=(),
            sim_require_finite=True,
            sim_require_nnan=True,
            nc=nc,
        )
        return tuple(outs)

    def _body_iters(iters):
        def f(*args):
            ins = args[:n_params]
            bufs = list(args[n_params:n_params + n_outs])
            for _ in range(iters):
                bufs = list(_call_once(ins, bufs))
            return tuple(bufs)
        return f

    _body = _body_iters(1)

    devices = jax.devices()[:8]
    mesh = Mesh(np.asarray(devices), ("core",))
    in_specs = (PartitionSpec("core"),) * (n_params + n_outs)
    out_specs = (PartitionSpec("core"),) * n_outs
    donate = tuple(range(n_params, n_params + n_outs))
    sharded = jax.jit(
        shard_map(_body, mesh=mesh, in_specs=in_specs, out_specs=out_specs,
                  check_rep=False),
        donate_argnums=donate, keep_unused=True)

    def _concat_in(in_maps):
        return [np.concatenate([np.asarray(m[name]) for m in in_maps], axis=0)
                for name in in_names]

    def _concat_zeros():
        return [np.zeros((8 * s[0], *s[1:]), d) for s, d in zero_shapes]

    def run(in_maps):
        out_arrs = sharded(*_concat_in(in_maps), *_concat_zeros())
        return [
            {name: np.asarray(out_arrs[i]).reshape(8, *out_avals[i].shape)[c]
             for i, name in enumerate(out_names)}
            for c in range(8)
        ]

    def make_timer(in_maps, iters=1):
        """Returns sample() -> wall seconds of `iters` chained executions
        (each feeding its output buffers into the next, async dispatch,
        one final sync), device-resident I/O."""
        import time as _time
        dev_in = [jax.device_put(x) for x in _concat_in(in_maps)]
        fn = jax.jit(
            shard_map(_body, mesh=mesh, in_specs=in_specs,
                      out_specs=out_specs, check_rep=False),
            donate_argnums=tuple(range(n_params, n_params + n_outs)),
            keep_unused=True)
        zz = [jax.device_put(z) for z in _concat_zeros()]
        bufs = fn(*dev_in, *zz)          # compile + warm
        jax.block_until_ready(bufs)

        state = {"bufs": bufs}

        def sample():
            bufs = state["bufs"]
            t0 = _time.time()
            for _ in range(iters):
                bufs = fn(*dev_in, *bufs)
            jax.block_until_ready(bufs)
            dt = _time.time() - t0
            state["bufs"] = bufs
            return dt
        return sample

    def time_exec(in_maps, repeats=10, iters=1):
        s = make_timer(in_maps, iters=iters)
        return min(s() for _ in range(repeats)) / iters

    run.time_exec = time_exec
    run.make_timer = make_timer
    _CACHE[key] = run
    return run


def baseline_time(repeats=10, iters=1):
    """Time an (almost) empty program with the same input/output signature
    as the kernel program, to subtract dispatch/RPC/input-registration
    overhead from time_exec (the axon execute path has a per-input-byte
    cost that is launch overhead, not kernel work)."""
    if ("baseline", iters) in _CACHE:
        return _CACHE[("baseline", iters)](repeats)
    import jax
    import concourse.tile as tile
    from concourse import bacc, mybir

    F16 = mybir.dt.float16
    nrows = int(os.environ.get("BK_NROWS", HALF))
    _, tot16 = _blob16_layout(nrows)
    nc = bacc.Bacc("TRN2", target_bir_lowering=False, debug=False, num_devices=8)
    x_e = nc.dram_tensor("blob16", [tot16], F16, kind="ExternalInput")
    out_e = nc.dram_tensor("out", [64, nrows, 512], F16, kind="ExternalOutput")
    with tile.TileContext(nc) as tc:
        with tc.tile_pool(name="sb", bufs=1) as sb:
            t = sb.tile([64, 512], F16)
            nc.sync.dma_start(t[:], x_e[0:64 * 512].rearrange("(r c) -> r c",
                                                              c=512))
            t2 = sb.tile([64, 512], F16)
            nc.scalar.copy(t2[:], t[:])
            nc.sync.dma_start(out_e[:, 0, :], t2[:])
    nc.compile()
    runner = _wrap_exec(nc, iters=iters)

    import numpy as _np
    in_maps = [{"blob16": _np.zeros(tot16, _np.float16)} for _ in range(8)]
    sampler = runner(in_maps)

    def bt(reps):
        return min(sampler() for _ in range(reps)) / iters

    bt.sample = sampler
    _CACHE[("baseline", iters)] = bt
    return bt(repeats)


def _wrap_exec(nc, iters=1):
    """Minimal timed executor for an arbitrary compiled nc (used by baseline)."""
    import jax
    import concourse.mybir as mybir
    from jax.sharding import Mesh, PartitionSpec
    from jax.experimental.shard_map import shard_map
    from concourse import bass2jax
    from concourse.bass2jax import _bass_exec_p, install_neuronx_cc_hook
    install_neuronx_cc_hook()

    part_name = nc.partition_id_tensor.name if nc.partition_id_tensor else None
    in_names, out_names, out_avals, zero_shapes = [], [], [], []
    for alloc in nc.m.functions[0].allocations:
        if not isinstance(alloc, mybir.MemoryLocationSet):
            continue
        name = alloc.memorylocations[0].name
        if alloc.kind == "ExternalInput":
            if name != part_name:
                in_names.append(name)
        elif alloc.kind == "ExternalOutput":
            shape = tuple(alloc.tensor_shape)
            dtype = mybir.dt.np(alloc.dtype)
            out_names.append(name)
            out_avals.append(jax.core.ShapedArray(shape, dtype))
            zero_shapes.append((shape, dtype))
    n_params, n_outs = len(in_names), len(out_names)
    all_names = in_names + out_names + ([part_name] if part_name else [])

    def _body(*args):
        operands = list(args)
        if part_name is not None:
            operands.append(bass2jax.partition_id_tensor())
        return tuple(_bass_exec_p.bind(
            *operands, out_avals=tuple(out_avals), in_names=tuple(all_names),
            out_names=tuple(out_names), lowering_input_output_aliases=(),
            sim_require_finite=True, sim_require_nnan=True, nc=nc))

    devices = jax.devices()[:8]
    mesh = Mesh(np.asarray(devices), ("core",))
    fn = jax.jit(
        shard_map(_body, mesh=mesh,
                  in_specs=(PartitionSpec("core"),) * (n_params + n_outs),
                  out_specs=(PartitionSpec("core"),) * n_outs,
                  check_rep=False),
        donate_argnums=tuple(range(n_params, n_params + n_outs)),
        keep_unused=True)

    def timed(in_maps):
        import time as _time
        dev_in = [jax.device_put(
            np.concatenate([np.asarray(m[nm]) for m in in_maps], axis=0))
            for nm in in_names]
        zz = [jax.device_put(np.zeros((8 * s[0], *s[1:]), d))
              for s, d in zero_shapes]
        bufs = fn(*dev_in, *zz)
        jax.block_until_ready(bufs)
        state = {"bufs": bufs}

        def sample():
            bufs = state["bufs"]
            t0 = _time.time()
            for _ in range(iters):
                bufs = fn(*dev_in, *bufs)
            jax.block_until_ready(bufs)
            dt = _time.time() - t0
            state["bufs"] = bufs
            return dt
        return sample

    return timed


def _make_in_maps(inputs, nrows):
    nx = np.asarray(inputs["nx"], np.float32)        # [4, 1, 512, 512]
    gks = [np.asarray(inputs[f"gk{i}"], np.float32) for i in range(4)]
    gfs, gwe, gwo = _build_g_matrices(gks, nrows)
    packs_even = [_pack_weights(inputs[f"w{l}"], False) for l in range(4)]
    packs_odd = [_pack_weights(inputs[f"w{l}"], True) for l in range(4)]
    lay16, tot16 = _blob16_layout(nrows)

    ba = np.zeros((128, 16), np.float16)
    for l in range(4):
        b2 = np.concatenate([np.asarray(inputs[f"b{l}"], np.float32)] * 2)
        ba[:, 2 * l:2 * l + 2] = b2.view(np.float16).reshape(128, 2)
        av = np.full(128, np.asarray(inputs[f"a{l}"], np.float32).reshape(1)[0],
                     np.float32)
        ba[:, 8 + 2 * l:10 + 2 * l] = av.view(np.float16).reshape(128, 2)

    in_maps = []
    for c in range(8):
        s, half = c >> 1, c & 1
        img = nx[s, 0]
        if half:
            img = img[::-1, :]
        vals = {"nx16": np.ascontiguousarray(img).astype(np.float16)}
        gw = gwo if half else gwe
        for i in range(4):
            vals[f"gw{i}"] = gw[i]
            vals[f"gf{i}"] = gfs[i]
        packs = packs_odd if half else packs_even
        for l in (1, 2, 3):
            vals[f"wy{l}"] = packs[l][0]
        for l in range(4):
            vals[f"wxa{l}"] = packs[l][1]
            vals[f"wxb{l}"] = packs[l][2]
        vals["ba"] = ba
        blob16 = np.zeros(tot16, np.float16)
        for name, (off, shp) in lay16.items():
            v = vals[name]
            assert v.shape == shp, (name, v.shape, shp)
            blob16[off:off + shp[0] * shp[1]] = v.ravel()
        in_maps.append({"blob16": blob16})
    return in_maps


def kernel(**inputs) -> np.ndarray:
    nrows = int(os.environ.get("BK_NROWS", HALF))
    rchunk = int(os.environ.get("BK_RCHUNK", 24))
    run = _get_exec(nrows, rchunk)
    in_maps = _make_in_maps(inputs, nrows)
    results = run(in_maps)
    out = np.zeros((B, 64, H, W), np.float32)
    for c in range(8):
        s, half = c >> 1, c & 1
        o = results[c]["out"].astype(np.float32)   # [64, nrows, 512] f16
        if half:
            out[s, :, H - nrows:H, :] = o[:, ::-1, :]
        else:
            out[s, :, 0:nrows, :] = o
    return out
